# revision 20
# baseline (speedup 1.0000x reference)
"""DeltaNet Trainium2 kernel (nn_DeltaNet_41961830482331).

Full module: qkv = x @ w_attn; per-(head,dim-group) standardization (ddof=1);
DeltaNet recurrence  S_t = S_{t-1}(0.99 I - 0.01 k k^T) + k v^T, o_t = S_t q_t;
y = o @ w_proj; out = x + y.

Sharding: 8 cores = 4 batches x 2 head-groups (6 heads each). Each core runs
the full pipeline for its (batch, head-group); host sums the two partial
y-projections per batch plus the residual x (w_proj is row-split across the
head-group pair).

Recurrence math (chunked, chunk n=128, gamma=0.99, beta=0.01):
substituting S_t = g^t Sh_t turns the decayed update into plain DeltaNet
  Sh_t = Sh_{t-1}(I - b' k k^T) + k nu_t^T,  b' = beta/g, nu_t = g^-t v_t,
  o_t = Sh_t qh_t, qh_t = g^t q_t.
Per chunk (K rows k_t, Vh rows nu_t, Qh rows qh_t, start state Sh0):
  N   = b' stril(K K^T)
  M   = (I + N)^{-1} (b'(stril(K Vh^T) K + K Sh0^T))
  O   = tril(Qh Vh^T) K - tril(Qh K^T) M + Qh Sh0^T
  Shn = Sh0 + K^T Vh - M^T K ;  next Sh0 = g^n Shn
The triangular solve uses the exact-through-N^7 factorization
  (I + N)^{-1} ~= (I - N)(I + N^2)(I + N^4)
with (I+N^4)v computed as v + N2(N2 v); signs are folded so the result is
-M directly.

Perf structure (vs the first version):
 - software pipeline one chunk deeper: prework (transposes/grams/masks/N^2)
   of chunk c+1 and the qkv projection+stats+norms of chunk c+2 are emitted
   inside chunk c's recurrence chain, so the PE queue never drains.
 - stats via bn_stats (one DVE op per qkv block) instead of the group-sum
   matmul path; mean/var of the two 32-element halves are merged exactly.
 - masks applied in two wide DVE ops per chunk over a 3-bank bf16 gram PSUM
   tile (masks pre-scaled by -b' on host, broadcast along the head dim).
 - solve intermediates are bf16 PSUM (single-shot matmuls), doubling DVE
   evac throughput; accumulating PSUMs (proj/R/O/state) stay f32.
 - ~40 dummy matmuls pre-warm the PE HAM clock gate during the DMA lead-in.
 - residual add moved to host; y stored bf16.
"""

import numpy as np

B, T, C = 4, 1024, 768
NH, HS = 12, 64
HPC = NH // 2            # heads per core
GAMMA, BETA = 0.99, 0.01
BP = BETA / GAMMA        # beta'
NC_ = 128                # chunk length n
NCH = T // NC_           # chunks
GN = GAMMA ** NC_        # gamma^n
W3 = 3 * HPC * HS        # 1152
KT = C // 128            # 6 contraction tiles for qkv proj
KP = HPC * HS // 128     # 3 contraction tiles for out proj
NWARM = 40               # HAM pre-warm matmuls

_cache: dict = {}


def _build_program():
    import concourse.bass as bass
    import concourse.tile as tile
    from concourse import bacc, mybir

    f32 = mybir.dt.float32
    bf16 = mybir.dt.bfloat16
    Alu = mybir.AluOpType
    Act = mybir.ActivationFunctionType

    nc = bacc.Bacc()

    # ---- DRAM parameters (per-core data; SPMD: same names on all cores) ----
    xT = nc.dram_tensor("xT", [C, T], bf16, kind="ExternalInput")          # x[b].T
    wA = nc.dram_tensor("wA", [C, W3], bf16, kind="ExternalInput")
    wP = nc.dram_tensor("wP", [HPC * HS, C], bf16, kind="ExternalInput")
    # consts = [id | M1 | M0]; M1 = [-b'*SL | IU | SU | IU] (512), M0 = -b'*SU
    consts = nc.dram_tensor("consts", [128, 768], bf16, kind="ExternalInput")
    gfac = nc.dram_tensor("gfac", [128, 18], f32, kind="ExternalInput")
    y = nc.dram_tensor("y", [T, C], bf16, kind="ExternalOutput")

    with tile.TileContext(nc) as tc:
        with (
            tc.tile_pool(name="persist", bufs=1) as persist,
            tc.tile_pool(name="qkvp", bufs=2) as qkvp,
            tc.tile_pool(name="statp", bufs=2) as statp,
            tc.tile_pool(name="natp", bufs=3) as natp,
            tc.tile_pool(name="tp", bufs=2) as tp,
            tc.tile_pool(name="gramp", bufs=2) as gramp,
            tc.tile_pool(name="solvep", bufs=2) as solvep,
            tc.tile_pool(name="stp", bufs=2) as stp,
            tc.tile_pool(name="yp", bufs=2) as yp,
            tc.tile_pool(name="ps_a", bufs=2, space="PSUM") as ps_a,
            tc.tile_pool(name="ps_g", bufs=3, space="PSUM") as ps_g,
            tc.tile_pool(name="ps_s", bufs=3, space="PSUM") as ps_s,
        ):
            # ---- persistent operands; chunk-0 columns of xT staged first so
            # the first projection can start early ----
            xT_sb = persist.tile([128, KT, T], bf16)
            wA_sb = persist.tile([128, KT, W3], bf16)
            xTr = xT.rearrange("(k p) t -> p k t", p=128)
            wAr = wA.rearrange("(k p) j -> p k j", p=128)
            for k in range(KT):
                nc.sync.dma_start(out=xT_sb[:, k, 0:NC_], in_=xTr[:, k, 0:NC_])
                nc.sync.dma_start(out=wA_sb[:, k, :], in_=wAr[:, k, :])
            for k in range(KT):
                nc.sync.dma_start(out=xT_sb[:, k, NC_:T], in_=xTr[:, k, NC_:T])
            cs_sb = persist.tile([128, 768], bf16)
            nc.gpsimd.dma_start(out=cs_sb, in_=consts[:, :])
            gf_sb = persist.tile([128, 18], f32)
            nc.gpsimd.dma_start(out=gf_sb, in_=gfac[:, :])
            wP_sb = persist.tile([128, KP, C], bf16)
            nc.gpsimd.dma_start(out=wP_sb, in_=wP.rearrange("(k p) j -> p k j", p=128))

            id_sb = cs_sb[:, 0:128]
            m1 = cs_sb[:, 128:640]    # [-b'SL | IU | SU | IU]
            m0 = cs_sb[:, 640:768]    # -b'SU

            # O^T for the whole sequence: [ch=384, t=1024]
            outT_sb = persist.tile([128, KP, T], bf16)

            # (no HAM pre-warm: on this part the PE clock is pinned at 1.2 GHz
            # regardless of activity, so warm-up matmuls only delay the start)
            st_prev = stp.tile([128, 192], bf16)
            nc.vector.memset(st_prev, 0.0)

            BCOL = {"q": 0, "k": 384, "v": 768}
            GSL = {"q": slice(0, 6), "k": slice(6, 12), "v": slice(12, 18)}

            def bc3(ap2d, g, d):
                """[128, g] -> broadcast [128, g, d]"""
                return ap2d.rearrange("p (g o) -> p g o", o=1).to_broadcast((128, g, d))

            def bch(ap2d, h, d):
                """[128, d] mask -> broadcast [128, h, d] over heads"""
                return ap2d.rearrange("p (o c) -> p o c", o=1).to_broadcast((128, h, d))

            def emit_block_proj(c, st, bname):
                """One qkv block (384 cols): project + evacuate."""
                t0 = c * NC_
                c0 = BCOL[bname]
                qkv_sb = st["qkv_sb"]
                pp = ps_a.tile([128, 384], f32, tag="qkvps", name="pp")
                for k in range(KT):
                    nc.tensor.matmul(
                        pp[:, :],
                        lhsT=xT_sb[:, k, t0:t0 + 128],
                        rhs=wA_sb[:, k, c0:c0 + 384],
                        start=(k == 0), stop=(k == KT - 1),
                    )
                nc.scalar.copy(qkv_sb[:, c0:c0 + 384], pp[:, :])

            def emit_stats(c, st):
                """per-chunk stats over all 18 groups -> rs, bi [128,18]."""
                qkv_sb = st["qkv_sb"]
                qkv3 = qkv_sb.rearrange("p (g d) -> p g d", d=64)
                sq = statp.tile([128, 1152], bf16, tag="sq")
                nc.gpsimd.tensor_tensor(sq, qkv_sb, qkv_sb, op=Alu.mult)
                s2 = statp.tile([128, 18], f32, tag="s2")
                nc.vector.tensor_reduce(
                    s2, sq.rearrange("p (g d) -> p g d", d=64),
                    axis=mybir.AxisListType.X, op=Alu.add)
                s1 = statp.tile([128, 18], f32, tag="s1")
                nc.vector.tensor_reduce(
                    s1, qkv3, axis=mybir.AxisListType.X, op=Alu.add)
                t1 = statp.tile([128, 18], f32, tag="t1")
                nc.gpsimd.tensor_tensor(t1, s1, s1, op=Alu.mult)
                # M2 = s2 - s1^2/64 ; var_unbiased = M2/63
                m2 = statp.tile([128, 18], f32, tag="m2")
                nc.vector.scalar_tensor_tensor(
                    out=m2, in0=t1, scalar=-1.0 / 64.0, in1=s2,
                    op0=Alu.mult, op1=Alu.add)
                sd = statp.tile([128, 18], f32, tag="sd")
                nc.scalar.activation(sd, m2, Act.Sqrt, scale=1.0 / 63.0)
                rstd = statp.tile([128, 18], f32, tag="rstd")
                nc.vector.reciprocal(rstd, sd)
                rs = statp.tile([128, 18], f32, tag="rs")
                nc.vector.tensor_tensor(rs, rstd, gf_sb, op=Alu.mult)
                # bias = -mean*rs = (s1 * -1/64) * rs
                bi = statp.tile([128, 18], f32, tag="bi")
                nc.vector.scalar_tensor_tensor(
                    out=bi, in0=s1, scalar=-1.0 / 64.0, in1=rs,
                    op0=Alu.mult, op1=Alu.mult)
                st["rs"] = rs
                st["bi"] = bi

            def emit_norms(c, st):
                """normalize qkv -> per-block norm tiles [128, 384] (pairs).
                k on ACT (per-group activation), q/v on DVE (tensor_scalar)."""
                qkv_sb = st["qkv_sb"]
                rs, bi = st["rs"], st["bi"]
                for bname in ("k", "v", "q"):
                    c0 = BCOL[bname]
                    g0 = GSL[bname].start
                    nb = natp.tile([128, 384], bf16, tag=f"{bname}n",
                                   name=f"nb_{bname}")
                    for i in range(6):
                        src = qkv_sb[:, c0 + 64 * i:c0 + 64 * i + 64]
                        dst = nb[:, 64 * i:64 * i + 64]
                        if bname == "k":
                            nc.scalar.activation(
                                dst, src, Act.Identity,
                                bias=bi[:, g0 + i:g0 + i + 1],
                                scale=rs[:, g0 + i:g0 + i + 1])
                        else:
                            nc.vector.tensor_scalar(
                                out=dst, in0=src,
                                scalar1=rs[:, g0 + i:g0 + i + 1],
                                scalar2=bi[:, g0 + i:g0 + i + 1],
                                op0=Alu.mult, op1=Alu.add)
                    st[bname] = nb
                st["kp"] = [st["k"][:, 128 * p:128 * p + 128] for p in range(3)]
                st["vp"] = [st["v"][:, 128 * p:128 * p + 128] for p in range(3)]
                st["qp"] = [st["q"][:, 128 * p:128 * p + 128] for p in range(3)]

            def emit_qkv(c, st):
                emit_block_proj(c, st, "k")
                emit_block_proj(c, st, "v")
                emit_block_proj(c, st, "q")
                emit_stats(c, st)
                emit_norms(c, st)

            # ---- prework: state-independent per-chunk recurrence inputs ----
            def emit_transposes(st):
                tsb_l = []
                for p in range(HPC // 2):
                    tps = ps_s.tile([128, 384], bf16, tag="sm", name="tps")
                    nc.tensor.transpose(tps[:, 128:256], st["qp"][p], id_sb)
                    nc.tensor.transpose(tps[:, 256:384], st["vp"][p], id_sb)
                    nc.tensor.transpose(tps[:, 0:128], st["kp"][p], id_sb)
                    tsb = tp.tile([128, 384], bf16, tag=f"tsb{p}", name="tsb")
                    nc.scalar.copy(tsb[:, :], tps[:, :])
                    tsb_l.append(tsb)
                st["tsb"] = tsb_l

            def emit_grams(st):
                """gram products per head + fused DVE mask evac.
                gsb[i] = [N_up | N_low | F2T | HvT | FiT] (640) per head."""
                gsb = gramp.tile([128, HPC, 640], bf16, tag="gsb", name="gsb")
                for p in range(HPC // 2):
                    tsb = st["tsb"][p]
                    # alternate row-groups (po=0/64) between consecutive matmuls
                    # so the PE can pull the next LDWEIGHTS ahead and overlap
                    gps2 = []
                    for sub in range(2):
                        po = 64 * sub
                        gps = ps_g.tile([128, 512], f32, tag="gram", name="gps")
                        nc.tensor.matmul(gps[:, 0:256],
                                         lhsT=tsb[po:po + 64, 0:128],
                                         rhs=tsb[po:po + 64, 0:256],
                                         tile_position=(po, 0))
                        gps2.append(gps)
                    for sub in range(2):
                        po = 64 * sub
                        nc.tensor.matmul(gps2[sub][:, 256:512],
                                         lhsT=tsb[po:po + 64, 256:384],
                                         rhs=tsb[po:po + 64, 0:256],
                                         tile_position=(po, 0))
                    for sub in range(2):
                        i = 2 * p + sub
                        nc.vector.tensor_tensor(
                            gsb[:, i, 128:640], gps2[sub][:, 0:512], m1,
                            op=Alu.mult)
                        nc.vector.tensor_tensor(
                            gsb[:, i, 0:128], gps2[sub][:, 0:128], m0,
                            op=Alu.mult)
                st["gsb"] = gsb

            def emit_n2(st):
                """N2T[i] = (N^2)^T (lhsT form for the solve)."""
                gsb = st["gsb"]
                n2T_l = []
                for half in range(2):
                    n2ps = ps_s.tile([128, 384], f32, tag="sm", name="n2ps")
                    for j in range(3):
                        i = 3 * half + j
                        nc.tensor.matmul(n2ps[:, 128 * j:128 * j + 128],
                                         lhsT=gsb[:, i, 128:256],
                                         rhs=gsb[:, i, 0:128])
                    n2sb = solvep.tile([128, 384], bf16, tag=f"n2T{half}",
                                       name="n2sb")
                    nc.scalar.copy(n2sb[:, :], n2ps[:, :])
                    n2T_l.append(n2sb)
                st["n2T"] = n2T_l

            def emit_rec(c, cur, pre1, nxt2):
                """recurrence for chunk c; pre1 = prework target (chunk c+1),
                nxt2 = qkv target (chunk c+2). Both may be None."""
                nonlocal st_prev
                t0 = c * NC_
                knp_l, vnp_l = cur["kp"], cur["vp"]
                tsb_l, gsb, n2T_l = cur["tsb"], cur["gsb"], cur["n2T"]

                def n2mm(dst, src, acc=None):
                    """dst[:, 64i:64i+64] = N2T_i^T @ src_i (+ acc if given,
                    via one 384-wide identity matmul accumulated in PSUM)."""
                    for i in range(HPC):
                        # in acc mode only the FIRST matmul may clear the bank:
                        # start=True resets has_written for the whole bank, so a
                        # later full-width accumulate would overwrite the rest
                        nc.tensor.matmul(
                            dst[:, 64 * i:64 * i + 64],
                            lhsT=n2T_l[i // 3][:, 128 * (i % 3):128 * (i % 3) + 128],
                            rhs=src[:, 64 * i:64 * i + 64],
                            start=(acc is None or i == 0), stop=(acc is None),
                            skip_group_check=acc is not None)
                    if acc is not None:
                        nc.tensor.matmul(dst[:, 0:384], lhsT=id_sb, rhs=acc,
                                         start=False, stop=True,
                                         skip_group_check=True)

                # next-next chunk's k projection keeps the PE streak across the
                # iteration boundary (R waits on the st_prev evac)
                if nxt2 is not None:
                    emit_block_proj(c + 2, nxt2, "k")

                # --- R_raw = HvT^T K + K Sh0^T ; r = -BP * R_raw = -R ---
                rps = ps_s.tile([128, 384], f32, tag="sm", name="rps")
                for i in range(HPC):
                    p, sub = divmod(i, 2)
                    po = 64 * sub
                    nc.tensor.matmul(rps[:, 64 * i:64 * i + 64],
                                     lhsT=gsb[:, i, 384:512],
                                     rhs=knp_l[p][:, po:po + 64],
                                     start=True, stop=False)
                    nc.tensor.matmul(rps[:, 64 * i:64 * i + 64],
                                     lhsT=tsb_l[p][po:po + 64, 0:128],
                                     rhs=st_prev[po:po + 64, 64 * p:64 * p + 64],
                                     start=False, stop=True, tile_position=(po, 0))
                r_sb = solvep.tile([128, 384], bf16, tag="rsb", name="r_sb")
                nc.scalar.mul(r_sb[:, :], rps[:, :], -BP)

                if nxt2 is not None:
                    emit_block_proj(c + 2, nxt2, "v")

                # --- u1 = N R - R  (the +R fold is an identity matmul) ---
                nrps = ps_s.tile([128, 384], f32, tag="sm", name="nrps")
                for i in range(HPC):
                    nc.tensor.matmul(nrps[:, 64 * i:64 * i + 64],
                                     lhsT=gsb[:, i, 0:128],
                                     rhs=r_sb[:, 64 * i:64 * i + 64],
                                     start=(i == 0), stop=False,
                                     skip_group_check=True)
                nc.tensor.matmul(nrps[:, 0:384], lhsT=id_sb, rhs=r_sb[:, 0:384],
                                 start=False, stop=True, skip_group_check=True)
                u1 = solvep.tile([128, 384], bf16, tag="u1", name="u1")
                nc.scalar.copy(u1[:, :], nrps[:, :])

                if pre1 is not None:
                    emit_transposes(pre1)

                # --- v1 = (I+N^2) u1 ---
                vps = ps_s.tile([128, 384], f32, tag="sm", name="vps")
                n2mm(vps, u1, acc=u1[:, 0:384])
                v1 = solvep.tile([128, 384], bf16, tag="v1", name="v1")
                nc.scalar.copy(v1[:, :], vps[:, :])

                if pre1 is not None:
                    emit_grams(pre1)

                # --- mneg = v1 + N^2 (N^2 v1) = (I+N^4) v1 = -M ---
                tps2 = ps_s.tile([128, 384], f32, tag="sm", name="tps2")
                n2mm(tps2, v1)
                tmp = solvep.tile([128, 384], bf16, tag="tmp", name="tmp")
                nc.scalar.copy(tmp[:, :], tps2[:, :])

                mps = ps_s.tile([128, 384], f32, tag="sm", name="mps")
                n2mm(mps, tmp, acc=v1[:, 0:384])
                mneg = solvep.tile([128, 384], bf16, tag="mneg", name="mneg")
                nc.scalar.copy(mneg[:, :], mps[:, :])

                # stats/norms of chunk c+2 land at the DVE queue tail: nothing
                # downstream this iteration depends on them
                if nxt2 is not None:
                    emit_block_proj(c + 2, nxt2, "q")
                    emit_stats(c + 2, nxt2)
                    emit_norms(c + 2, nxt2)

                # --- O^T = K^T FiT + Sh0 Qh^T + Mneg^T F2T (one evac) ---
                o_ps = ps_s.tile([128, 384], f32, tag="sm", name="o_ps")
                for p in range(HPC // 2):
                    for sub in range(2):
                        i = 2 * p + sub
                        po = 64 * sub
                        sl = slice(po, po + 64)
                        ow = o_ps[sl, 128 * p:128 * p + 128]
                        nc.tensor.matmul(ow, lhsT=knp_l[p][:, sl],
                                         rhs=gsb[:, i, 512:640],
                                         start=True, stop=False, tile_position=(0, po))
                        nc.tensor.matmul(ow,
                                         lhsT=st_prev[sl, 64 * p:64 * p + 64],
                                         rhs=tsb_l[p][sl, 128:256],
                                         start=False, stop=False, tile_position=(po, po))
                        nc.tensor.matmul(ow,
                                         lhsT=mneg[:, 64 * i:64 * i + 64],
                                         rhs=gsb[:, i, 256:384],
                                         start=False, stop=True, tile_position=(0, po))
                nc.scalar.copy(
                    outT_sb[:, :, t0:t0 + 128],
                    o_ps.rearrange("p (k t) -> p k t", k=KP))

                # --- state: Shn^T = Sh0^T + Vh^T K + K^T Mneg ---
                sps = ps_s.tile([128, 192], f32, tag="sm", name="sps")
                for i in range(HPC):
                    p, sub = divmod(i, 2)
                    po = 64 * sub
                    psl = slice(po, po + 64)
                    fsl = slice(64 * p, 64 * p + 64)
                    nc.tensor.matmul(sps[psl, fsl], lhsT=vnp_l[p][:, psl],
                                     rhs=knp_l[p][:, psl],
                                     start=True, stop=False, tile_position=(0, po))
                    nc.tensor.matmul(sps[psl, fsl], lhsT=knp_l[p][:, psl],
                                     rhs=mneg[:, 64 * i:64 * i + 64],
                                     start=False, stop=False, tile_position=(0, po))
                    nc.tensor.matmul(sps[psl, fsl], lhsT=id_sb[psl, psl],
                                     rhs=st_prev[psl, fsl],
                                     start=False, stop=True, tile_position=(po, po))
                st_new = stp.tile([128, 192], bf16, name="st_new")
                nc.scalar.mul(st_new[:, :], sps[:, :], GN)
                st_prev = st_new

            def emit_proj(c):
                """output projection + store for chunk c (residual on host)."""
                t0 = c * NC_
                y_sb = yp.tile([128, C], bf16, tag="ysb", name="y_sb")
                for nblk in range(2):
                    ypp = ps_a.tile([128, 384], f32, tag="qkvps", name="ypp")
                    for k in range(KP):
                        nc.tensor.matmul(
                            ypp[:, :],
                            lhsT=outT_sb[:, k, t0:t0 + 128],
                            rhs=wP_sb[:, k, 384 * nblk:384 * nblk + 384],
                            start=(k == 0), stop=(k == KP - 1),
                        )
                    nc.scalar.copy(y_sb[:, 384 * nblk:384 * nblk + 384], ypp[:, :])
                nc.sync.dma_start(out=y[t0:t0 + 128, :], in_=y_sb[:, :])

            # ---- pipeline ----
            states = {}
            states[0] = {"qkv_sb": qkvp.tile([128, W3], bf16, tag="qkv", name="qkv0")}
            emit_qkv(0, states[0])
            states[1] = {"qkv_sb": qkvp.tile([128, W3], bf16, tag="qkv", name="qkv1")}
            emit_qkv(1, states[1])
            emit_transposes(states[0])
            emit_grams(states[0])
            emit_n2(states[0])
            for c in range(NCH):
                pre1 = states.get(c + 1)
                if c + 2 < NCH:
                    nxt2 = {"qkv_sb": qkvp.tile([128, W3], bf16, tag="qkv",
                                                name="qkvn")}
                    states[c + 2] = nxt2
                else:
                    nxt2 = None
                emit_rec(c, states[c], pre1, nxt2)
                emit_proj(c)
                if pre1 is not None:
                    emit_n2(pre1)
                del states[c]

    nc.finalize()
    return nc


def _host_inputs(x, w_attn, w_proj):
    """Build the 8 per-core input maps."""
    import ml_dtypes
    bf = ml_dtypes.bfloat16
    in_maps = []
    gfac = np.ones((128, 18), np.float32)
    p = np.arange(1, 129, dtype=np.float64)
    gfac[:, 0:6] = (GAMMA ** p)[:, None]
    gfac[:, 12:18] = (GAMMA ** (-p))[:, None]
    ii, jj = np.indices((128, 128))
    mSU = (jj > ii).astype(np.float32)
    mSL = (ii > jj).astype(np.float32)
    mIU = (jj >= ii).astype(np.float32)
    consts = np.concatenate([
        np.eye(128, dtype=np.float32),
        -BP * mSL, mIU, mSU, mIU,           # M1
        -BP * mSU,                          # M0
    ], axis=1).astype(bf)
    for core in range(8):
        b, hg = divmod(core, 2)
        h0 = hg * HPC
        cols = []
        for blk in range(3):   # q, k, v column blocks of w_attn
            cols.append(w_attn[:, blk * C + h0 * HS: blk * C + (h0 + HPC) * HS])
        wA_s = np.ascontiguousarray(np.concatenate(cols, axis=1)).astype(bf)
        wP_s = np.ascontiguousarray(w_proj[h0 * HS:(h0 + HPC) * HS]).astype(bf)
        xb = np.ascontiguousarray(x[b])                                # [1024, 768]
        in_maps.append({
            "xT": np.ascontiguousarray(xb.T).astype(bf),
            "wA": wA_s,
            "wP": wP_s,
            "gfac": gfac,
            "consts": consts,
        })
    return in_maps


def kernel(x, w_attn, w_proj):
    from concourse.bass_utils import run_bass_kernel_spmd

    if "nc" not in _cache:
        _cache["nc"] = _build_program()
    nc = _cache["nc"]

    x = np.asarray(x)
    in_maps = _host_inputs(x, np.asarray(w_attn), np.asarray(w_proj))
    res = run_bass_kernel_spmd(nc, in_maps, core_ids=list(range(8)))
    out = np.empty((B, T, C), np.float32)
    for b in range(B):
        out[b] = (res.results[2 * b]["y"].astype(np.float32)
                  + res.results[2 * b + 1]["y"].astype(np.float32)
                  + x[b])
    return out


# revision 21
# speedup vs baseline: 1.1793x; 1.1793x over previous
"""DeltaNet Trainium2 kernel (nn_DeltaNet_41961830482331).

Full module: qkv = x @ w_attn; per-(head,dim-group) standardization (ddof=1);
DeltaNet recurrence  S_t = S_{t-1}(0.99 I - 0.01 k k^T) + k v^T, o_t = S_t q_t;
y = o @ w_proj; out = x + y.

Sharding: 8 cores = 4 batches x 2 head-groups (6 heads each). Each core runs
the full pipeline for its (batch, head-group); host sums the two partial
y-projections per batch plus the residual x (w_proj is row-split across the
head-group pair).

Recurrence math (chunked, chunk n=128, gamma=0.99, beta=0.01):
substituting S_t = g^t Sh_t turns the decayed update into plain DeltaNet
  Sh_t = Sh_{t-1}(I - b' k k^T) + k nu_t^T,  b' = beta/g, nu_t = g^-t v_t,
  o_t = Sh_t qh_t, qh_t = g^t q_t.
Per chunk (K rows k_t, Vh rows nu_t, Qh rows qh_t, start state Sh0):
  N   = b' stril(K K^T)
  M   = (I + N)^{-1} (b'(stril(K Vh^T) K + K Sh0^T))
  O   = tril(Qh Vh^T) K - tril(Qh K^T) M + Qh Sh0^T
  Shn = Sh0 + K^T Vh - M^T K ;  next Sh0 = g^n Shn
The triangular solve uses the exact-through-N^7 factorization
  (I + N)^{-1} ~= (I - N)(I + N^2)(I + N^4)
with (I+N^4)v computed as v + N2(N2 v); signs are folded so the result is
-M directly.

Perf structure (vs the first version):
 - software pipeline one chunk deeper: prework (transposes/grams+masks/N^2)
   of chunk c+1 and the qkv projection+stats+norms of chunk c+2 are emitted
   inside chunk c's recurrence chain, so the PE queue never drains. Ordering
   is engine-FIFO-aware: the DVE queue per iteration holds only mask ops and
   stats/norms, neither of which the solve chain waits on.
 - the solve-chain adds (u1 = NR-R etc.) run on the PE as one 384-wide
   identity matmul accumulated into the same PSUM group (only the first
   region matmul carries start=True: start clears has_written for the whole
   bank), with plain ACT copies out - the DVE is fully off the critical path.
 - per-chunk stats (one Pool square + two wide DVE reduces over all 18
   groups) replace the group-sum matmul path; gamma powers folded into rs.
 - masks fused into the gram PSUM evac (two DVE ops per head, -b' pre-scaled
   into the mask constants); gram pair matmuls alternate row-groups so
   LDWEIGHTS pull-ahead overlaps them.
 - residual add moved to host; y stored bf16; no xres/wAg inputs.
Note: on this part the PE clock is pinned at 1.2 GHz (HAM never engages;
verified with a 6.5us continuous matmul stream), so PE cycle count is the
binding budget; whole-chip clock also varies ~20% run to run.
"""

import numpy as np

B, T, C = 4, 1024, 768
NH, HS = 12, 64
HPC = NH // 2            # heads per core
GAMMA, BETA = 0.99, 0.01
BP = BETA / GAMMA        # beta'
NC_ = 128                # chunk length n
NCH = T // NC_           # chunks
GN = GAMMA ** NC_        # gamma^n
W3 = 3 * HPC * HS        # 1152
KT = C // 128            # 6 contraction tiles for qkv proj
KP = HPC * HS // 128     # 3 contraction tiles for out proj
NWARM = 40               # HAM pre-warm matmuls

_cache: dict = {}


def _build_program():
    import concourse.bass as bass
    import concourse.tile as tile
    from concourse import bacc, mybir

    f32 = mybir.dt.float32
    bf16 = mybir.dt.bfloat16
    Alu = mybir.AluOpType
    Act = mybir.ActivationFunctionType

    nc = bacc.Bacc()

    # ---- DRAM parameters (per-core data; SPMD: same names on all cores) ----
    xT = nc.dram_tensor("xT", [C, T], bf16, kind="ExternalInput")          # x[b].T
    wA = nc.dram_tensor("wA", [C, W3], bf16, kind="ExternalInput")
    wP = nc.dram_tensor("wP", [HPC * HS, C], bf16, kind="ExternalInput")
    # consts = [id | M1 | M0]; M1 = [-b'*SL | IU | SU | IU] (512), M0 = -b'*SU
    consts = nc.dram_tensor("consts", [128, 768], bf16, kind="ExternalInput")
    gfac = nc.dram_tensor("gfac", [128, 18], f32, kind="ExternalInput")
    y = nc.dram_tensor("y", [T, C], bf16, kind="ExternalOutput")

    with tile.TileContext(nc) as tc:
        with (
            tc.tile_pool(name="persist", bufs=1) as persist,
            tc.tile_pool(name="qkvp", bufs=2) as qkvp,
            tc.tile_pool(name="statp", bufs=2) as statp,
            tc.tile_pool(name="natp", bufs=3) as natp,
            tc.tile_pool(name="tp", bufs=2) as tp,
            tc.tile_pool(name="gramp", bufs=2) as gramp,
            tc.tile_pool(name="solvep", bufs=2) as solvep,
            tc.tile_pool(name="stp", bufs=2) as stp,
            tc.tile_pool(name="yp", bufs=2) as yp,
            tc.tile_pool(name="ps_a", bufs=2, space="PSUM") as ps_a,
            tc.tile_pool(name="ps_g", bufs=3, space="PSUM") as ps_g,
            tc.tile_pool(name="ps_s", bufs=3, space="PSUM") as ps_s,
        ):
            # ---- persistent operands; chunk-0 columns of xT staged first so
            # the first projection can start early ----
            xT_sb = persist.tile([128, KT, T], bf16)
            wA_sb = persist.tile([128, KT, W3], bf16)
            xTr = xT.rearrange("(k p) t -> p k t", p=128)
            wAr = wA.rearrange("(k p) j -> p k j", p=128)
            for k in range(KT):
                nc.sync.dma_start(out=xT_sb[:, k, 0:NC_], in_=xTr[:, k, 0:NC_])
                nc.sync.dma_start(out=wA_sb[:, k, :], in_=wAr[:, k, :])
            for k in range(KT):
                nc.sync.dma_start(out=xT_sb[:, k, NC_:T], in_=xTr[:, k, NC_:T])
            cs_sb = persist.tile([128, 768], bf16)
            nc.gpsimd.dma_start(out=cs_sb, in_=consts[:, :])
            gf_sb = persist.tile([128, 18], f32)
            nc.gpsimd.dma_start(out=gf_sb, in_=gfac[:, :])
            wP_sb = persist.tile([128, KP, C], bf16)
            nc.gpsimd.dma_start(out=wP_sb, in_=wP.rearrange("(k p) j -> p k j", p=128))

            id_sb = cs_sb[:, 0:128]
            m1 = cs_sb[:, 128:640]    # [-b'SL | IU | SU | IU]
            m0 = cs_sb[:, 640:768]    # -b'SU

            # O^T for the whole sequence: [ch=384, t=1024]
            outT_sb = persist.tile([128, KP, T], bf16)

            # (no HAM pre-warm: on this part the PE clock is pinned at 1.2 GHz
            # regardless of activity, so warm-up matmuls only delay the start)
            st_prev = stp.tile([128, 192], bf16)
            nc.vector.memset(st_prev, 0.0)

            BCOL = {"q": 0, "k": 384, "v": 768}
            GSL = {"q": slice(0, 6), "k": slice(6, 12), "v": slice(12, 18)}

            def bc3(ap2d, g, d):
                """[128, g] -> broadcast [128, g, d]"""
                return ap2d.rearrange("p (g o) -> p g o", o=1).to_broadcast((128, g, d))

            def bch(ap2d, h, d):
                """[128, d] mask -> broadcast [128, h, d] over heads"""
                return ap2d.rearrange("p (o c) -> p o c", o=1).to_broadcast((128, h, d))

            def emit_block_proj(c, st, bname):
                """One qkv block (384 cols): project + evacuate."""
                t0 = c * NC_
                c0 = BCOL[bname]
                qkv_sb = st["qkv_sb"]
                pp = ps_a.tile([128, 384], f32, tag="qkvps", name="pp")
                for k in range(KT):
                    nc.tensor.matmul(
                        pp[:, :],
                        lhsT=xT_sb[:, k, t0:t0 + 128],
                        rhs=wA_sb[:, k, c0:c0 + 384],
                        start=(k == 0), stop=(k == KT - 1),
                    )
                nc.scalar.copy(qkv_sb[:, c0:c0 + 384], pp[:, :])

            def emit_stats(c, st):
                """per-chunk stats over all 18 groups -> rs, bi [128,18]."""
                qkv_sb = st["qkv_sb"]
                qkv3 = qkv_sb.rearrange("p (g d) -> p g d", d=64)
                sq = statp.tile([128, 1152], bf16, tag="sq")
                nc.gpsimd.tensor_tensor(sq, qkv_sb, qkv_sb, op=Alu.mult)
                s2 = statp.tile([128, 18], f32, tag="s2")
                nc.vector.tensor_reduce(
                    s2, sq.rearrange("p (g d) -> p g d", d=64),
                    axis=mybir.AxisListType.X, op=Alu.add)
                s1 = statp.tile([128, 18], f32, tag="s1")
                nc.vector.tensor_reduce(
                    s1, qkv3, axis=mybir.AxisListType.X, op=Alu.add)
                t1 = statp.tile([128, 18], f32, tag="t1")
                nc.gpsimd.tensor_tensor(t1, s1, s1, op=Alu.mult)
                # M2 = s2 - s1^2/64 ; var_unbiased = M2/63
                m2 = statp.tile([128, 18], f32, tag="m2")
                nc.vector.scalar_tensor_tensor(
                    out=m2, in0=t1, scalar=-1.0 / 64.0, in1=s2,
                    op0=Alu.mult, op1=Alu.add)
                sd = statp.tile([128, 18], f32, tag="sd")
                nc.scalar.activation(sd, m2, Act.Sqrt, scale=1.0 / 63.0)
                rstd = statp.tile([128, 18], f32, tag="rstd")
                nc.vector.reciprocal(rstd, sd)
                rs = statp.tile([128, 18], f32, tag="rs")
                nc.vector.tensor_tensor(rs, rstd, gf_sb, op=Alu.mult)
                # bias = -mean*rs = (s1 * -1/64) * rs
                bi = statp.tile([128, 18], f32, tag="bi")
                nc.vector.scalar_tensor_tensor(
                    out=bi, in0=s1, scalar=-1.0 / 64.0, in1=rs,
                    op0=Alu.mult, op1=Alu.mult)
                st["rs"] = rs
                st["bi"] = bi

            def emit_norms(c, st):
                """normalize qkv -> per-block norm tiles [128, 384] (pairs).
                k on ACT (per-group activation), q/v on DVE (tensor_scalar)."""
                qkv_sb = st["qkv_sb"]
                rs, bi = st["rs"], st["bi"]
                for bname in ("k", "v", "q"):
                    c0 = BCOL[bname]
                    g0 = GSL[bname].start
                    nb = natp.tile([128, 384], bf16, tag=f"{bname}n",
                                   name=f"nb_{bname}")
                    for i in range(6):
                        src = qkv_sb[:, c0 + 64 * i:c0 + 64 * i + 64]
                        dst = nb[:, 64 * i:64 * i + 64]
                        if bname == "k":
                            nc.scalar.activation(
                                dst, src, Act.Identity,
                                bias=bi[:, g0 + i:g0 + i + 1],
                                scale=rs[:, g0 + i:g0 + i + 1])
                        else:
                            nc.vector.tensor_scalar(
                                out=dst, in0=src,
                                scalar1=rs[:, g0 + i:g0 + i + 1],
                                scalar2=bi[:, g0 + i:g0 + i + 1],
                                op0=Alu.mult, op1=Alu.add)
                    st[bname] = nb
                st["kp"] = [st["k"][:, 128 * p:128 * p + 128] for p in range(3)]
                st["vp"] = [st["v"][:, 128 * p:128 * p + 128] for p in range(3)]
                st["qp"] = [st["q"][:, 128 * p:128 * p + 128] for p in range(3)]

            def emit_qkv(c, st):
                emit_block_proj(c, st, "k")
                emit_block_proj(c, st, "v")
                emit_block_proj(c, st, "q")
                emit_stats(c, st)
                emit_norms(c, st)

            # ---- prework: state-independent per-chunk recurrence inputs ----
            def emit_transposes(st):
                tsb_l = []
                for p in range(HPC // 2):
                    tps = ps_s.tile([128, 384], bf16, tag="sm", name="tps")
                    nc.tensor.transpose(tps[:, 128:256], st["qp"][p], id_sb)
                    nc.tensor.transpose(tps[:, 256:384], st["vp"][p], id_sb)
                    nc.tensor.transpose(tps[:, 0:128], st["kp"][p], id_sb)
                    tsb = tp.tile([128, 384], bf16, tag=f"tsb{p}", name="tsb")
                    nc.scalar.copy(tsb[:, :], tps[:, :])
                    tsb_l.append(tsb)
                st["tsb"] = tsb_l

            def emit_grams(st):
                """gram products per head + fused DVE mask evac.
                gsb[i] = [N_up | N_low | F2T | HvT | FiT] (640) per head."""
                gsb = gramp.tile([128, HPC, 640], bf16, tag="gsb", name="gsb")
                for p in range(HPC // 2):
                    tsb = st["tsb"][p]
                    # alternate row-groups (po=0/64) between consecutive matmuls
                    # so the PE can pull the next LDWEIGHTS ahead and overlap
                    gps2 = []
                    for sub in range(2):
                        po = 64 * sub
                        gps = ps_g.tile([128, 512], f32, tag="gram", name="gps")
                        nc.tensor.matmul(gps[:, 0:256],
                                         lhsT=tsb[po:po + 64, 0:128],
                                         rhs=tsb[po:po + 64, 0:256],
                                         tile_position=(po, 0))
                        gps2.append(gps)
                    for sub in range(2):
                        po = 64 * sub
                        nc.tensor.matmul(gps2[sub][:, 256:512],
                                         lhsT=tsb[po:po + 64, 256:384],
                                         rhs=tsb[po:po + 64, 0:256],
                                         tile_position=(po, 0))
                    for sub in range(2):
                        i = 2 * p + sub
                        nc.vector.tensor_tensor(
                            gsb[:, i, 128:640], gps2[sub][:, 0:512], m1,
                            op=Alu.mult)
                        nc.vector.tensor_tensor(
                            gsb[:, i, 0:128], gps2[sub][:, 0:128], m0,
                            op=Alu.mult)
                st["gsb"] = gsb

            def emit_n2(st):
                """N2T[i] = (N^2)^T (lhsT form for the solve)."""
                gsb = st["gsb"]
                n2T_l = []
                for half in range(2):
                    n2ps = ps_s.tile([128, 384], f32, tag="sm", name="n2ps")
                    for j in range(3):
                        i = 3 * half + j
                        nc.tensor.matmul(n2ps[:, 128 * j:128 * j + 128],
                                         lhsT=gsb[:, i, 128:256],
                                         rhs=gsb[:, i, 0:128])
                    n2sb = solvep.tile([128, 384], bf16, tag=f"n2T{half}",
                                       name="n2sb")
                    nc.scalar.copy(n2sb[:, :], n2ps[:, :])
                    n2T_l.append(n2sb)
                st["n2T"] = n2T_l

            def emit_rec(c, cur, pre1, nxt2):
                """recurrence for chunk c; pre1 = prework target (chunk c+1),
                nxt2 = qkv target (chunk c+2). Both may be None."""
                nonlocal st_prev
                t0 = c * NC_
                knp_l, vnp_l = cur["kp"], cur["vp"]
                tsb_l, gsb, n2T_l = cur["tsb"], cur["gsb"], cur["n2T"]

                def n2mm(dst, src, acc=None):
                    """dst[:, 64i:64i+64] = N2T_i^T @ src_i (+ acc if given,
                    via one 384-wide identity matmul accumulated in PSUM)."""
                    for i in range(HPC):
                        # in acc mode only the FIRST matmul may clear the bank:
                        # start=True resets has_written for the whole bank, so a
                        # later full-width accumulate would overwrite the rest
                        nc.tensor.matmul(
                            dst[:, 64 * i:64 * i + 64],
                            lhsT=n2T_l[i // 3][:, 128 * (i % 3):128 * (i % 3) + 128],
                            rhs=src[:, 64 * i:64 * i + 64],
                            start=(acc is None or i == 0), stop=(acc is None),
                            skip_group_check=acc is not None)
                    if acc is not None:
                        nc.tensor.matmul(dst[:, 0:384], lhsT=id_sb, rhs=acc,
                                         start=False, stop=True,
                                         skip_group_check=True)

                # next-next chunk's k projection keeps the PE streak across the
                # iteration boundary (R waits on the st_prev evac)
                if nxt2 is not None:
                    emit_block_proj(c + 2, nxt2, "k")

                # --- R_raw = HvT^T K + K Sh0^T ; r = -BP * R_raw = -R ---
                rps = ps_s.tile([128, 384], f32, tag="sm", name="rps")
                for i in range(HPC):
                    p, sub = divmod(i, 2)
                    po = 64 * sub
                    nc.tensor.matmul(rps[:, 64 * i:64 * i + 64],
                                     lhsT=gsb[:, i, 384:512],
                                     rhs=knp_l[p][:, po:po + 64],
                                     start=True, stop=False)
                    nc.tensor.matmul(rps[:, 64 * i:64 * i + 64],
                                     lhsT=tsb_l[p][po:po + 64, 0:128],
                                     rhs=st_prev[po:po + 64, 64 * p:64 * p + 64],
                                     start=False, stop=True, tile_position=(po, 0))
                r_sb = solvep.tile([128, 384], bf16, tag="rsb", name="r_sb")
                nc.scalar.mul(r_sb[:, :], rps[:, :], -BP)

                if nxt2 is not None:
                    emit_block_proj(c + 2, nxt2, "v")

                # --- u1 = N R - R  (the +R fold is an identity matmul) ---
                nrps = ps_s.tile([128, 384], f32, tag="sm", name="nrps")
                for i in range(HPC):
                    nc.tensor.matmul(nrps[:, 64 * i:64 * i + 64],
                                     lhsT=gsb[:, i, 0:128],
                                     rhs=r_sb[:, 64 * i:64 * i + 64],
                                     start=(i == 0), stop=False,
                                     skip_group_check=True)
                nc.tensor.matmul(nrps[:, 0:384], lhsT=id_sb, rhs=r_sb[:, 0:384],
                                 start=False, stop=True, skip_group_check=True)
                u1 = solvep.tile([128, 384], bf16, tag="u1", name="u1")
                nc.scalar.copy(u1[:, :], nrps[:, :])

                if pre1 is not None:
                    emit_transposes(pre1)

                # --- v1 = (I+N^2) u1 ---
                vps = ps_s.tile([128, 384], f32, tag="sm", name="vps")
                n2mm(vps, u1, acc=u1[:, 0:384])
                v1 = solvep.tile([128, 384], bf16, tag="v1", name="v1")
                nc.scalar.copy(v1[:, :], vps[:, :])

                if pre1 is not None:
                    emit_grams(pre1)

                # --- mneg = v1 + N^2 (N^2 v1) = (I+N^4) v1 = -M ---
                tps2 = ps_s.tile([128, 384], f32, tag="sm", name="tps2")
                n2mm(tps2, v1)
                tmp = solvep.tile([128, 384], bf16, tag="tmp", name="tmp")
                nc.scalar.copy(tmp[:, :], tps2[:, :])

                mps = ps_s.tile([128, 384], f32, tag="sm", name="mps")
                n2mm(mps, tmp, acc=v1[:, 0:384])
                mneg = solvep.tile([128, 384], bf16, tag="mneg", name="mneg")
                nc.scalar.copy(mneg[:, :], mps[:, :])

                # stats/norms of chunk c+2 land at the DVE queue tail: nothing
                # downstream this iteration depends on them
                if nxt2 is not None:
                    emit_block_proj(c + 2, nxt2, "q")
                    emit_stats(c + 2, nxt2)
                    emit_norms(c + 2, nxt2)

                # --- O^T = K^T FiT + Sh0 Qh^T + Mneg^T F2T (one evac) ---
                o_ps = ps_s.tile([128, 384], f32, tag="sm", name="o_ps")
                for p in range(HPC // 2):
                    for sub in range(2):
                        i = 2 * p + sub
                        po = 64 * sub
                        sl = slice(po, po + 64)
                        ow = o_ps[sl, 128 * p:128 * p + 128]
                        nc.tensor.matmul(ow, lhsT=knp_l[p][:, sl],
                                         rhs=gsb[:, i, 512:640],
                                         start=True, stop=False, tile_position=(0, po))
                        nc.tensor.matmul(ow,
                                         lhsT=st_prev[sl, 64 * p:64 * p + 64],
                                         rhs=tsb_l[p][sl, 128:256],
                                         start=False, stop=False, tile_position=(po, po))
                        nc.tensor.matmul(ow,
                                         lhsT=mneg[:, 64 * i:64 * i + 64],
                                         rhs=gsb[:, i, 256:384],
                                         start=False, stop=True, tile_position=(0, po))
                nc.scalar.copy(
                    outT_sb[:, :, t0:t0 + 128],
                    o_ps.rearrange("p (k t) -> p k t", k=KP))

                # --- state: Shn^T = Sh0^T + Vh^T K + K^T Mneg ---
                sps = ps_s.tile([128, 192], f32, tag="sm", name="sps")
                for i in range(HPC):
                    p, sub = divmod(i, 2)
                    po = 64 * sub
                    psl = slice(po, po + 64)
                    fsl = slice(64 * p, 64 * p + 64)
                    nc.tensor.matmul(sps[psl, fsl], lhsT=vnp_l[p][:, psl],
                                     rhs=knp_l[p][:, psl],
                                     start=True, stop=False, tile_position=(0, po))
                    nc.tensor.matmul(sps[psl, fsl], lhsT=knp_l[p][:, psl],
                                     rhs=mneg[:, 64 * i:64 * i + 64],
                                     start=False, stop=False, tile_position=(0, po))
                    nc.tensor.matmul(sps[psl, fsl], lhsT=id_sb[psl, psl],
                                     rhs=st_prev[psl, fsl],
                                     start=False, stop=True, tile_position=(po, po))
                st_new = stp.tile([128, 192], bf16, name="st_new")
                nc.scalar.mul(st_new[:, :], sps[:, :], GN)
                st_prev = st_new

            def emit_proj(c):
                """output projection + store for chunk c (residual on host)."""
                t0 = c * NC_
                y_sb = yp.tile([128, C], bf16, tag="ysb", name="y_sb")
                for nblk in range(2):
                    ypp = ps_a.tile([128, 384], f32, tag="qkvps", name="ypp")
                    for k in range(KP):
                        nc.tensor.matmul(
                            ypp[:, :],
                            lhsT=outT_sb[:, k, t0:t0 + 128],
                            rhs=wP_sb[:, k, 384 * nblk:384 * nblk + 384],
                            start=(k == 0), stop=(k == KP - 1),
                        )
                    nc.scalar.copy(y_sb[:, 384 * nblk:384 * nblk + 384], ypp[:, :])
                nc.sync.dma_start(out=y[t0:t0 + 128, :], in_=y_sb[:, :])

            # ---- pipeline ----
            states = {}
            states[0] = {"qkv_sb": qkvp.tile([128, W3], bf16, tag="qkv", name="qkv0")}
            emit_qkv(0, states[0])
            states[1] = {"qkv_sb": qkvp.tile([128, W3], bf16, tag="qkv", name="qkv1")}
            emit_qkv(1, states[1])
            emit_transposes(states[0])
            emit_grams(states[0])
            emit_n2(states[0])
            for c in range(NCH):
                pre1 = states.get(c + 1)
                if c + 2 < NCH:
                    nxt2 = {"qkv_sb": qkvp.tile([128, W3], bf16, tag="qkv",
                                                name="qkvn")}
                    states[c + 2] = nxt2
                else:
                    nxt2 = None
                emit_rec(c, states[c], pre1, nxt2)
                emit_proj(c)
                if pre1 is not None:
                    emit_n2(pre1)
                del states[c]

    nc.finalize()
    return nc


def _host_inputs(x, w_attn, w_proj):
    """Build the 8 per-core input maps."""
    import ml_dtypes
    bf = ml_dtypes.bfloat16
    in_maps = []
    gfac = np.ones((128, 18), np.float32)
    p = np.arange(1, 129, dtype=np.float64)
    gfac[:, 0:6] = (GAMMA ** p)[:, None]
    gfac[:, 12:18] = (GAMMA ** (-p))[:, None]
    ii, jj = np.indices((128, 128))
    mSU = (jj > ii).astype(np.float32)
    mSL = (ii > jj).astype(np.float32)
    mIU = (jj >= ii).astype(np.float32)
    consts = np.concatenate([
        np.eye(128, dtype=np.float32),
        -BP * mSL, mIU, mSU, mIU,           # M1
        -BP * mSU,                          # M0
    ], axis=1).astype(bf)
    for core in range(8):
        b, hg = divmod(core, 2)
        h0 = hg * HPC
        cols = []
        for blk in range(3):   # q, k, v column blocks of w_attn
            cols.append(w_attn[:, blk * C + h0 * HS: blk * C + (h0 + HPC) * HS])
        wA_s = np.ascontiguousarray(np.concatenate(cols, axis=1)).astype(bf)
        wP_s = np.ascontiguousarray(w_proj[h0 * HS:(h0 + HPC) * HS]).astype(bf)
        xb = np.ascontiguousarray(x[b])                                # [1024, 768]
        in_maps.append({
            "xT": np.ascontiguousarray(xb.T).astype(bf),
            "wA": wA_s,
            "wP": wP_s,
            "gfac": gfac,
            "consts": consts,
        })
    return in_maps


def kernel(x, w_attn, w_proj):
    from concourse.bass_utils import run_bass_kernel_spmd

    if "nc" not in _cache:
        _cache["nc"] = _build_program()
    nc = _cache["nc"]

    x = np.asarray(x)
    in_maps = _host_inputs(x, np.asarray(w_attn), np.asarray(w_proj))
    res = run_bass_kernel_spmd(nc, in_maps, core_ids=list(range(8)))
    out = np.empty((B, T, C), np.float32)
    for b in range(B):
        out[b] = (res.results[2 * b]["y"].astype(np.float32)
                  + res.results[2 * b + 1]["y"].astype(np.float32)
                  + x[b])
    return out


# revision 28
# speedup vs baseline: 1.2076x; 1.0240x over previous
"""DeltaNet Trainium2 kernel (nn_DeltaNet_41961830482331).

Full module: qkv = x @ w_attn; per-(head,dim-group) standardization (ddof=1);
DeltaNet recurrence  S_t = S_{t-1}(0.99 I - 0.01 k k^T) + k v^T, o_t = S_t q_t;
y = o @ w_proj; out = x + y.

Sharding: 8 cores = 4 batches x 2 head-groups (6 heads each). Each core runs
the full pipeline for its (batch, head-group); host sums the two partial
y-projections per batch plus the residual x (w_proj is row-split across the
head-group pair).

Recurrence math (chunked, chunk n=128, gamma=0.99, beta=0.01):
substituting S_t = g^t Sh_t turns the decayed update into plain DeltaNet
  Sh_t = Sh_{t-1}(I - b' k k^T) + k nu_t^T,  b' = beta/g, nu_t = g^-t v_t,
  o_t = Sh_t qh_t, qh_t = g^t q_t.
Per chunk (K rows k_t, Vh rows nu_t, Qh rows qh_t, start state Sh0):
  N   = b' stril(K K^T)
  M   = (I + N)^{-1} (b'(stril(K Vh^T) K + K Sh0^T))
  O   = tril(Qh Vh^T) K - tril(Qh K^T) M + Qh Sh0^T
  Shn = Sh0 + K^T Vh - M^T K ;  next Sh0 = g^n Shn
The triangular solve uses the exact-through-N^7 factorization
  (I + N)^{-1} ~= (I - N)(I + N^2)(I + N^4)
with (I+N^4)v computed as v + N2(N2 v); signs are folded so the result is
-M directly.

Perf structure (vs the first version):
 - software pipeline one chunk deeper: prework (transposes/grams+masks/N^2)
   of chunk c+1 and the qkv projection+stats+norms of chunk c+2 are emitted
   inside chunk c's recurrence chain, so the PE queue never drains. Ordering
   is engine-FIFO-aware: the DVE queue per iteration holds only mask ops and
   stats/norms, neither of which the solve chain waits on.
 - the solve-chain adds (u1 = NR-R etc.) run on the PE as one 384-wide
   identity matmul accumulated into the same PSUM group (only the first
   region matmul carries start=True: start clears has_written for the whole
   bank), with plain ACT copies out - the DVE is fully off the critical path.
 - per-chunk stats (one Pool square + two wide DVE reduces over all 18
   groups) replace the group-sum matmul path; gamma powers folded into rs.
 - masks fused into the gram PSUM evac (two DVE ops per head, -b' pre-scaled
   into the mask constants); gram pair matmuls alternate row-groups so
   LDWEIGHTS pull-ahead overlaps them.
 - residual add moved to host; y stored bf16; no xres/wAg inputs.
Note: on this part the PE clock is pinned at 1.2 GHz (HAM never engages;
verified with a 6.5us continuous matmul stream), so PE cycle count is the
binding budget; whole-chip clock also varies ~20% run to run.
"""

import numpy as np

B, T, C = 4, 1024, 768
NH, HS = 12, 64
HPC = NH // 2            # heads per core
GAMMA, BETA = 0.99, 0.01
BP = BETA / GAMMA        # beta'
NC_ = 128                # chunk length n
NCH = T // NC_           # chunks
GN = GAMMA ** NC_        # gamma^n
W3 = 3 * HPC * HS        # 1152
KT = C // 128            # 6 contraction tiles for qkv proj
KP = HPC * HS // 128     # 3 contraction tiles for out proj

_cache: dict = {}


def _build_program():
    import concourse.bass as bass
    import concourse.tile as tile
    from concourse import bacc, mybir

    f32 = mybir.dt.float32
    bf16 = mybir.dt.bfloat16
    Alu = mybir.AluOpType
    Act = mybir.ActivationFunctionType

    nc = bacc.Bacc()

    # ---- DRAM parameters (per-core data; SPMD: same names on all cores) ----
    xT = nc.dram_tensor("xT", [C, T], bf16, kind="ExternalInput")          # x[b].T
    wA = nc.dram_tensor("wA", [C, W3], bf16, kind="ExternalInput")
    wP = nc.dram_tensor("wP", [HPC * HS, C], bf16, kind="ExternalInput")
    # consts = [id | M1 | M0]; M1 = [-b'*SL | IU | SU | IU] (512), M0 = -b'*SU
    consts = nc.dram_tensor("consts", [128, 768], bf16, kind="ExternalInput")
    gfac = nc.dram_tensor("gfac", [128, 18], f32, kind="ExternalInput")
    y = nc.dram_tensor("y", [T, C], bf16, kind="ExternalOutput")

    with tile.TileContext(nc) as tc:
        with (
            tc.tile_pool(name="persist", bufs=1) as persist,
            tc.tile_pool(name="qkvp", bufs=2) as qkvp,
            tc.tile_pool(name="statp", bufs=2) as statp,
            tc.tile_pool(name="natp", bufs=3) as natp,
            tc.tile_pool(name="tp", bufs=2) as tp,
            tc.tile_pool(name="gramp", bufs=2) as gramp,
            tc.tile_pool(name="solvep", bufs=2) as solvep,
            tc.tile_pool(name="stp", bufs=2) as stp,
            tc.tile_pool(name="yp", bufs=2) as yp,
            tc.tile_pool(name="ps_a", bufs=2, space="PSUM") as ps_a,
            tc.tile_pool(name="ps_g", bufs=3, space="PSUM") as ps_g,
            tc.tile_pool(name="ps_s", bufs=3, space="PSUM") as ps_s,
        ):
            # ---- persistent operands; chunk-0 columns of xT staged first so
            # the first projection can start early ----
            xT_sb = persist.tile([128, KT, T], bf16)
            wA_sb = persist.tile([128, KT, W3], bf16)
            xTr = xT.rearrange("(k p) t -> p k t", p=128)
            wAr = wA.rearrange("(k p) j -> p k j", p=128)
            # stage DMA in consumption order: chunk-0/1 x columns and the
            # k-block of wA first so the first projection starts ASAP
            for k in range(KT):
                nc.sync.dma_start(out=xT_sb[:, k, 0:2 * NC_], in_=xTr[:, k, 0:2 * NC_])
                nc.sync.dma_start(out=wA_sb[:, k, 384:768], in_=wAr[:, k, 384:768])
            for k in range(KT):
                nc.sync.dma_start(out=wA_sb[:, k, 768:1152], in_=wAr[:, k, 768:1152])
            for k in range(KT):
                nc.sync.dma_start(out=wA_sb[:, k, 0:384], in_=wAr[:, k, 0:384])
            for k in range(KT):
                nc.sync.dma_start(out=xT_sb[:, k, 2 * NC_:T], in_=xTr[:, k, 2 * NC_:T])
            cs_sb = persist.tile([128, 768], bf16)
            nc.gpsimd.dma_start(out=cs_sb, in_=consts[:, :])
            gf_sb = persist.tile([128, 18], f32)
            nc.gpsimd.dma_start(out=gf_sb, in_=gfac[:, :])
            wP_sb = persist.tile([128, KP, C], bf16)
            nc.gpsimd.dma_start(out=wP_sb, in_=wP.rearrange("(k p) j -> p k j", p=128))

            id_sb = cs_sb[:, 0:128]
            m1 = cs_sb[:, 128:640]    # [-b'SL | IU | SU | IU]
            m0 = cs_sb[:, 640:768]    # -b'SU

            # O^T for the whole sequence: [ch=384, t=1024]
            outT_sb = persist.tile([128, KP, T], bf16)

            # (no HAM pre-warm: on this part the PE clock is pinned at 1.2 GHz
            # regardless of activity, so warm-up matmuls only delay the start)
            st_prev = stp.tile([128, 192], bf16)
            nc.vector.memset(st_prev, 0.0)

            BCOL = {"q": 0, "k": 384, "v": 768}
            GSL = {"q": slice(0, 6), "k": slice(6, 12), "v": slice(12, 18)}

            def bc3(ap2d, g, d):
                """[128, g] -> broadcast [128, g, d]"""
                return ap2d.rearrange("p (g o) -> p g o", o=1).to_broadcast((128, g, d))

            def bch(ap2d, h, d):
                """[128, d] mask -> broadcast [128, h, d] over heads"""
                return ap2d.rearrange("p (o c) -> p o c", o=1).to_broadcast((128, h, d))

            def emit_block_proj(c, st, bname):
                """One qkv block (384 cols): project + evacuate."""
                t0 = c * NC_
                c0 = BCOL[bname]
                qkv_sb = st["qkv_sb"]
                pp = ps_a.tile([128, 384], f32, tag="qkvps", name="pp")
                for k in range(KT):
                    nc.tensor.matmul(
                        pp[:, :],
                        lhsT=xT_sb[:, k, t0:t0 + 128],
                        rhs=wA_sb[:, k, c0:c0 + 384],
                        start=(k == 0), stop=(k == KT - 1),
                    )
                nc.scalar.copy(qkv_sb[:, c0:c0 + 384], pp[:, :])

            def emit_stats(c, st):
                """per-chunk stats over all 18 groups -> rs, bi [128,18]."""
                qkv_sb = st["qkv_sb"]
                qkv3 = qkv_sb.rearrange("p (g d) -> p g d", d=64)
                sq = statp.tile([128, 1152], bf16, tag="sq")
                nc.gpsimd.tensor_tensor(sq, qkv_sb, qkv_sb, op=Alu.mult)
                s2 = statp.tile([128, 18], f32, tag="s2")
                nc.vector.tensor_reduce(
                    s2, sq.rearrange("p (g d) -> p g d", d=64),
                    axis=mybir.AxisListType.X, op=Alu.add)
                s1 = statp.tile([128, 18], f32, tag="s1")
                nc.vector.tensor_reduce(
                    s1, qkv3, axis=mybir.AxisListType.X, op=Alu.add)
                t1 = statp.tile([128, 18], f32, tag="t1")
                nc.gpsimd.tensor_tensor(t1, s1, s1, op=Alu.mult)
                # M2 = s2 - s1^2/64 ; var_unbiased = M2/63
                m2 = statp.tile([128, 18], f32, tag="m2")
                nc.vector.scalar_tensor_tensor(
                    out=m2, in0=t1, scalar=-1.0 / 64.0, in1=s2,
                    op0=Alu.mult, op1=Alu.add)
                sd = statp.tile([128, 18], f32, tag="sd")
                nc.scalar.activation(sd, m2, Act.Sqrt, scale=1.0 / 63.0)
                rstd = statp.tile([128, 18], f32, tag="rstd")
                nc.vector.reciprocal(rstd, sd)
                rs = statp.tile([128, 18], f32, tag="rs")
                nc.vector.tensor_tensor(rs, rstd, gf_sb, op=Alu.mult)
                # bias = -mean*rs = (s1 * -1/64) * rs
                bi = statp.tile([128, 18], f32, tag="bi")
                nc.vector.scalar_tensor_tensor(
                    out=bi, in0=s1, scalar=-1.0 / 64.0, in1=rs,
                    op0=Alu.mult, op1=Alu.mult)
                st["rs"] = rs
                st["bi"] = bi

            def emit_norms(c, st):
                """normalize qkv -> per-block norm tiles [128, 384] (pairs).
                k on ACT (per-group activation), q/v on DVE (tensor_scalar)."""
                qkv_sb = st["qkv_sb"]
                rs, bi = st["rs"], st["bi"]
                for bname in ("k", "v", "q"):
                    c0 = BCOL[bname]
                    g0 = GSL[bname].start
                    nb = natp.tile([128, 384], bf16, tag=f"{bname}n",
                                   name=f"nb_{bname}")
                    for i in range(6):
                        src = qkv_sb[:, c0 + 64 * i:c0 + 64 * i + 64]
                        dst = nb[:, 64 * i:64 * i + 64]
                        if bname == "k":
                            nc.scalar.activation(
                                dst, src, Act.Identity,
                                bias=bi[:, g0 + i:g0 + i + 1],
                                scale=rs[:, g0 + i:g0 + i + 1])
                        else:
                            nc.vector.tensor_scalar(
                                out=dst, in0=src,
                                scalar1=rs[:, g0 + i:g0 + i + 1],
                                scalar2=bi[:, g0 + i:g0 + i + 1],
                                op0=Alu.mult, op1=Alu.add)
                    st[bname] = nb
                st["kp"] = [st["k"][:, 128 * p:128 * p + 128] for p in range(3)]
                st["vp"] = [st["v"][:, 128 * p:128 * p + 128] for p in range(3)]
                st["qp"] = [st["q"][:, 128 * p:128 * p + 128] for p in range(3)]

            def emit_qkv(c, st):
                emit_block_proj(c, st, "k")
                emit_block_proj(c, st, "v")
                emit_block_proj(c, st, "q")
                emit_stats(c, st)
                emit_norms(c, st)

            # ---- prework: state-independent per-chunk recurrence inputs ----
            def emit_transposes(st):
                tsb_l = []
                for p in range(HPC // 2):
                    tps = ps_s.tile([128, 384], bf16, tag="sm", name="tps")
                    nc.tensor.transpose(tps[:, 128:256], st["qp"][p], id_sb)
                    nc.tensor.transpose(tps[:, 256:384], st["vp"][p], id_sb)
                    nc.tensor.transpose(tps[:, 0:128], st["kp"][p], id_sb)
                    tsb = tp.tile([128, 384], bf16, tag=f"tsb{p}", name="tsb")
                    nc.scalar.copy(tsb[:, :], tps[:, :])
                    tsb_l.append(tsb)
                st["tsb"] = tsb_l

            def emit_gram_pair(st, p):
                """gram products for head pair p + fused DVE mask evac.
                gsb[i] = [N_up | N_low | F2T | HvT | FiT] (640) per head."""
                if "gsb" not in st:
                    st["gsb"] = gramp.tile([128, HPC, 640], bf16, tag="gsb",
                                           name="gsb")
                gsb = st["gsb"]
                if True:
                    tsb = st["tsb"][p]
                    # alternate row-groups (po=0/64) between consecutive matmuls
                    # so the PE can pull the next LDWEIGHTS ahead and overlap
                    gps2 = []
                    for sub in range(2):
                        po = 64 * sub
                        gps = ps_g.tile([128, 512], f32, tag="gram", name="gps")
                        nc.tensor.matmul(gps[:, 0:256],
                                         lhsT=tsb[po:po + 64, 0:128],
                                         rhs=tsb[po:po + 64, 0:256],
                                         tile_position=(po, 0))
                        gps2.append(gps)
                    for sub in range(2):
                        po = 64 * sub
                        nc.tensor.matmul(gps2[sub][:, 256:512],
                                         lhsT=tsb[po:po + 64, 256:384],
                                         rhs=tsb[po:po + 64, 0:256],
                                         tile_position=(po, 0))
                    for sub in range(2):
                        i = 2 * p + sub
                        nc.vector.tensor_tensor(
                            gsb[:, i, 128:640], gps2[sub][:, 0:512], m1,
                            op=Alu.mult)
                        nc.vector.tensor_tensor(
                            gsb[:, i, 0:128], gps2[sub][:, 0:128], m0,
                            op=Alu.mult)

            def emit_grams(st):
                for p in range(HPC // 2):
                    emit_gram_pair(st, p)

            def emit_n2(st):
                """N2T[i] = (N^2)^T (lhsT form for the solve)."""
                gsb = st["gsb"]
                n2T_l = []
                for half in range(2):
                    n2ps = ps_s.tile([128, 384], f32, tag="sm", name="n2ps")
                    for j in range(3):
                        i = 3 * half + j
                        nc.tensor.matmul(n2ps[:, 128 * j:128 * j + 128],
                                         lhsT=gsb[:, i, 128:256],
                                         rhs=gsb[:, i, 0:128])
                    n2sb = solvep.tile([128, 384], bf16, tag=f"n2T{half}",
                                       name="n2sb")
                    nc.scalar.copy(n2sb[:, :], n2ps[:, :])
                    n2T_l.append(n2sb)
                st["n2T"] = n2T_l

            def emit_rec(c, cur, pre1, nxt2):
                """recurrence for chunk c; pre1 = prework target (chunk c+1),
                nxt2 = qkv target (chunk c+2). Both may be None."""
                nonlocal st_prev
                t0 = c * NC_
                knp_l, vnp_l = cur["kp"], cur["vp"]
                tsb_l, gsb, n2T_l = cur["tsb"], cur["gsb"], cur["n2T"]

                def n2mm(dst, src, acc=None):
                    """dst[:, 64i:64i+64] = N2T_i^T @ src_i (+ acc if given,
                    via one 384-wide identity matmul accumulated in PSUM)."""
                    for i in range(HPC):
                        # in acc mode only the FIRST matmul may clear the bank:
                        # start=True resets has_written for the whole bank, so a
                        # later full-width accumulate would overwrite the rest
                        nc.tensor.matmul(
                            dst[:, 64 * i:64 * i + 64],
                            lhsT=n2T_l[i // 3][:, 128 * (i % 3):128 * (i % 3) + 128],
                            rhs=src[:, 64 * i:64 * i + 64],
                            start=(acc is None or i == 0), stop=(acc is None),
                            skip_group_check=acc is not None)
                    if acc is not None:
                        nc.tensor.matmul(dst[:, 0:384], lhsT=id_sb, rhs=acc,
                                         start=False, stop=True,
                                         skip_group_check=True)

                # next-next chunk's k projection keeps the PE streak across the
                # iteration boundary (R waits on the st_prev evac)
                if nxt2 is not None:
                    emit_block_proj(c + 2, nxt2, "k")

                # --- R_raw = HvT^T K + K Sh0^T ; r = -BP * R_raw = -R ---
                rps = ps_s.tile([128, 384], f32, tag="sm", name="rps")
                for i in range(HPC):
                    p, sub = divmod(i, 2)
                    po = 64 * sub
                    nc.tensor.matmul(rps[:, 64 * i:64 * i + 64],
                                     lhsT=gsb[:, i, 384:512],
                                     rhs=knp_l[p][:, po:po + 64],
                                     start=True, stop=False)
                    nc.tensor.matmul(rps[:, 64 * i:64 * i + 64],
                                     lhsT=tsb_l[p][po:po + 64, 0:128],
                                     rhs=st_prev[po:po + 64, 64 * p:64 * p + 64],
                                     start=False, stop=True, tile_position=(po, 0))
                r_sb = solvep.tile([128, 384], bf16, tag="rsb", name="r_sb")
                nc.scalar.mul(r_sb[:, :], rps[:, :], -BP)

                if nxt2 is not None:
                    emit_block_proj(c + 2, nxt2, "v")

                # --- u1 = N R - R  (the +R fold is an identity matmul) ---
                nrps = ps_s.tile([128, 384], f32, tag="sm", name="nrps")
                for i in range(HPC):
                    nc.tensor.matmul(nrps[:, 64 * i:64 * i + 64],
                                     lhsT=gsb[:, i, 0:128],
                                     rhs=r_sb[:, 64 * i:64 * i + 64],
                                     start=(i == 0), stop=False,
                                     skip_group_check=True)
                nc.tensor.matmul(nrps[:, 0:384], lhsT=id_sb, rhs=r_sb[:, 0:384],
                                 start=False, stop=True, skip_group_check=True)
                u1 = solvep.tile([128, 384], bf16, tag="u1", name="u1")
                nc.scalar.copy(u1[:, :], nrps[:, :])

                if pre1 is not None:
                    emit_transposes(pre1)

                # --- v1 = (I+N^2) u1 ---
                vps = ps_s.tile([128, 384], f32, tag="sm", name="vps")
                n2mm(vps, u1, acc=u1[:, 0:384])
                v1 = solvep.tile([128, 384], bf16, tag="v1", name="v1")
                nc.scalar.copy(v1[:, :], vps[:, :])

                # gram pairs are spread between the solve passes: their mask
                # ops drain on the DVE while the PE alternates pass/gram work
                if pre1 is not None:
                    emit_gram_pair(pre1, 0)

                # --- mneg = v1 + N^2 (N^2 v1) = (I+N^4) v1 = -M ---
                tps2 = ps_s.tile([128, 384], f32, tag="sm", name="tps2")
                n2mm(tps2, v1)
                tmp = solvep.tile([128, 384], bf16, tag="tmp", name="tmp")
                nc.scalar.copy(tmp[:, :], tps2[:, :])

                if pre1 is not None:
                    emit_gram_pair(pre1, 1)
                if nxt2 is not None:
                    emit_block_proj(c + 2, nxt2, "q")

                mps = ps_s.tile([128, 384], f32, tag="sm", name="mps")
                n2mm(mps, tmp, acc=v1[:, 0:384])
                mneg = solvep.tile([128, 384], bf16, tag="mneg", name="mneg")
                nc.scalar.copy(mneg[:, :], mps[:, :])

                if pre1 is not None:
                    emit_gram_pair(pre1, 2)

                # stats/norms of chunk c+2 land at the DVE queue tail: nothing
                # downstream this iteration depends on them
                if nxt2 is not None:
                    emit_stats(c + 2, nxt2)
                    emit_norms(c + 2, nxt2)

                # --- O^T = K^T FiT + Sh0 Qh^T + Mneg^T F2T (one evac) ---
                o_ps = ps_s.tile([128, 384], f32, tag="sm", name="o_ps")
                for p in range(HPC // 2):
                    for sub in range(2):
                        i = 2 * p + sub
                        po = 64 * sub
                        sl = slice(po, po + 64)
                        ow = o_ps[sl, 128 * p:128 * p + 128]
                        nc.tensor.matmul(ow, lhsT=knp_l[p][:, sl],
                                         rhs=gsb[:, i, 512:640],
                                         start=True, stop=False, tile_position=(0, po))
                        nc.tensor.matmul(ow,
                                         lhsT=st_prev[sl, 64 * p:64 * p + 64],
                                         rhs=tsb_l[p][sl, 128:256],
                                         start=False, stop=False, tile_position=(po, po))
                        nc.tensor.matmul(ow,
                                         lhsT=mneg[:, 64 * i:64 * i + 64],
                                         rhs=gsb[:, i, 256:384],
                                         start=False, stop=True, tile_position=(0, po))
                nc.scalar.copy(
                    outT_sb[:, :, t0:t0 + 128],
                    o_ps.rearrange("p (k t) -> p k t", k=KP))

                # --- state: Shn^T = Sh0^T + Vh^T K + K^T Mneg ---
                sps = ps_s.tile([128, 192], f32, tag="sm", name="sps")
                for i in range(HPC):
                    p, sub = divmod(i, 2)
                    po = 64 * sub
                    psl = slice(po, po + 64)
                    fsl = slice(64 * p, 64 * p + 64)
                    nc.tensor.matmul(sps[psl, fsl], lhsT=vnp_l[p][:, psl],
                                     rhs=knp_l[p][:, psl],
                                     start=True, stop=False, tile_position=(0, po))
                    nc.tensor.matmul(sps[psl, fsl], lhsT=knp_l[p][:, psl],
                                     rhs=mneg[:, 64 * i:64 * i + 64],
                                     start=False, stop=False, tile_position=(0, po))
                    nc.tensor.matmul(sps[psl, fsl], lhsT=id_sb[psl, psl],
                                     rhs=st_prev[psl, fsl],
                                     start=False, stop=True, tile_position=(po, po))
                st_new = stp.tile([128, 192], bf16, name="st_new")
                nc.scalar.mul(st_new[:, :], sps[:, :], GN)
                st_prev = st_new

            def emit_proj(c):
                """output projection + store for chunk c (residual on host)."""
                t0 = c * NC_
                y_sb = yp.tile([128, C], bf16, tag="ysb", name="y_sb")
                for nblk in range(2):
                    # out-proj uses the gram PSUM pool so it does not couple
                    # into the qkv-projection PSUM rotation
                    ypp = ps_g.tile([128, 512], f32, tag="gram", name="ypp")[:, 0:384]
                    for k in range(KP):
                        nc.tensor.matmul(
                            ypp[:, :],
                            lhsT=outT_sb[:, k, t0:t0 + 128],
                            rhs=wP_sb[:, k, 384 * nblk:384 * nblk + 384],
                            start=(k == 0), stop=(k == KP - 1),
                        )
                    nc.scalar.copy(y_sb[:, 384 * nblk:384 * nblk + 384], ypp[:, :])
                nc.sync.dma_start(out=y[t0:t0 + 128, :], in_=y_sb[:, :])

            # ---- pipeline ----
            # startup: chunk-1 projections separate the chunk-0 stats chain
            # from chunk-0 prework on the PE; chunk-0 masks precede chunk-1
            # stats in the DVE FIFO so the first recurrence starts sooner
            states = {}
            states[0] = {"qkv_sb": qkvp.tile([128, W3], bf16, tag="qkv", name="qkv0")}
            emit_qkv(0, states[0])
            states[1] = {"qkv_sb": qkvp.tile([128, W3], bf16, tag="qkv", name="qkv1")}
            for bname in ("k", "v", "q"):
                emit_block_proj(1, states[1], bname)
            emit_transposes(states[0])
            emit_grams(states[0])
            emit_stats(1, states[1])
            emit_norms(1, states[1])
            emit_n2(states[0])
            for c in range(NCH):
                pre1 = states.get(c + 1)
                if c + 2 < NCH:
                    nxt2 = {"qkv_sb": qkvp.tile([128, W3], bf16, tag="qkv",
                                                name="qkvn")}
                    states[c + 2] = nxt2
                else:
                    nxt2 = None
                emit_rec(c, states[c], pre1, nxt2)
                emit_proj(c)
                if pre1 is not None:
                    emit_n2(pre1)
                del states[c]

    nc.finalize()
    return nc


def _host_inputs(x, w_attn, w_proj):
    """Build the 8 per-core input maps."""
    import ml_dtypes
    bf = ml_dtypes.bfloat16
    in_maps = []
    gfac = np.ones((128, 18), np.float32)
    p = np.arange(1, 129, dtype=np.float64)
    gfac[:, 0:6] = (GAMMA ** p)[:, None]
    gfac[:, 12:18] = (GAMMA ** (-p))[:, None]
    ii, jj = np.indices((128, 128))
    mSU = (jj > ii).astype(np.float32)
    mSL = (ii > jj).astype(np.float32)
    mIU = (jj >= ii).astype(np.float32)
    consts = np.concatenate([
        np.eye(128, dtype=np.float32),
        -BP * mSL, mIU, mSU, mIU,           # M1
        -BP * mSU,                          # M0
    ], axis=1).astype(bf)
    for core in range(8):
        b, hg = divmod(core, 2)
        h0 = hg * HPC
        cols = []
        for blk in range(3):   # q, k, v column blocks of w_attn
            cols.append(w_attn[:, blk * C + h0 * HS: blk * C + (h0 + HPC) * HS])
        wA_s = np.ascontiguousarray(np.concatenate(cols, axis=1)).astype(bf)
        wP_s = np.ascontiguousarray(w_proj[h0 * HS:(h0 + HPC) * HS]).astype(bf)
        xb = np.ascontiguousarray(x[b])                                # [1024, 768]
        in_maps.append({
            "xT": np.ascontiguousarray(xb.T).astype(bf),
            "wA": wA_s,
            "wP": wP_s,
            "gfac": gfac,
            "consts": consts,
        })
    return in_maps


def kernel(x, w_attn, w_proj):
    from concourse.bass_utils import run_bass_kernel_spmd

    if "nc" not in _cache:
        _cache["nc"] = _build_program()
    nc = _cache["nc"]

    x = np.asarray(x)
    in_maps = _host_inputs(x, np.asarray(w_attn), np.asarray(w_proj))
    res = run_bass_kernel_spmd(nc, in_maps, core_ids=list(range(8)))
    out = np.empty((B, T, C), np.float32)
    for b in range(B):
        out[b] = (res.results[2 * b]["y"].astype(np.float32)
                  + res.results[2 * b + 1]["y"].astype(np.float32)
                  + x[b])
    return out
